# revision 21
# baseline (speedup 1.0000x reference)
"""AngularDistribution Trainium2 kernel (8 NeuronCores, SPMD data-parallel over (batch,atom) pairs).

Math (per pair p, triple n, offset r, filter f):
  rad[n,r]  = exp(c_r*S1[n] - 12*o_r^2 - g*S2[n])     c_r = 2*g*o_r, S1=rij+rik+rjk, S2=sum sq
  ang[n,f]  = 2*u^z (f<4, u=(1-ct)/2) or 2*v^z (f>=4, v=(1+ct)/2), z in {1,2,4,8}
  cut[n]    = (cos(pi*rij/10)*cos(pi*rik/10)*cos(pi*rjk/10))^2
  out[p,r*8+f] = sum_n rad[n,r] * ang[n,f] * cut[n] * mask[n]

The full exponent u (always <= 0) is computed on the TensorEngine as one matmul per
pair: lhsT = interleaved transposed rows [S1(9) ; S2g(9) ; ones(1)] (K=2*nch+1, M=128),
rhs = constant block-diagonal [K, nch*32] carrying c_r blocks, -1 blocks and the
-12*o_r^2 row.  exp reads the PSUM result directly.  Masked triples are compacted
host-side; padding r=5.0 gives cut ~= 0 exactly like the reference's (r<5) gate.
"""

import os
import sys

sys.path.insert(0, "/opt/trn_rl_repo")

import numpy as np
from contextlib import ExitStack

GAMMA = 4.0
N_CORES = 8
PP = 64          # pairs per core (512 total / 8)
R = 32
F = 8

_CACHE = {}
LAST_EXEC_NS = None


def _build(nch):
    import concourse.bass as bass
    import concourse.tile as tile
    from concourse.tile import add_dep_helper
    from concourse import bacc, mybir

    f32 = mybir.dt.float32
    Alu = mybir.AluOpType
    Act = mybir.ActivationFunctionType
    NPAD = nch * 128
    W = PP * nch          # global tile free size
    KR = 5 * nch + 2      # lhsT rows/pair: S1hi,S1lo,S1hi(for clo),S2ghi,S2glo,ones*2
    KRP = 64              # padded row stride: 2 pairs per 128-row block
    NV = PP * KRP         # virtual rows in transposed source
    NB = NV // 128        # 128-col blocks in SRC / TS

    nc = bacc.Bacc("TRN2", target_bir_lowering=False, debug=False,
                   num_devices=N_CORES)

    d_rij = nc.dram_tensor("rij", [PP, NPAD], f32, kind="ExternalInput")
    d_rik = nc.dram_tensor("rik", [PP, NPAD], f32, kind="ExternalInput")
    d_rjk = nc.dram_tensor("rjk", [PP, NPAD], f32, kind="ExternalInput")
    d_bd = nc.dram_tensor("bdiag", [128, nch * R], mybir.dt.bfloat16, kind="ExternalInput")
    d_id = nc.dram_tensor("ident", [128, 128], mybir.dt.bfloat16, kind="ExternalInput")
    d_sel = nc.dram_tensor("sel", [128, R], f32, kind="ExternalInput")
    d_out = nc.dram_tensor("out", [PP, R * F], f32, kind="ExternalOutput")

    with tile.TileContext(nc) as tc, ExitStack() as ctx:
        cpool = ctx.enter_context(tc.tile_pool(name="consts", bufs=1))
        gpool = ctx.enter_context(tc.tile_pool(name="glob", bufs=1))
        ppool = ctx.enter_context(tc.tile_pool(name="pair", bufs=4))
        pcpool = ctx.enter_context(tc.tile_pool(name="pc", bufs=1, space="PSUM"))
        ps2pool = ctx.enter_context(tc.tile_pool(name="ps2", bufs=1, space="PSUM"))
        pupool = ctx.enter_context(tc.tile_pool(name="psu", bufs=3, space="PSUM"))
        ptpool = ps2pool

        bf16c = mybir.dt.bfloat16
        bd_t = cpool.tile([128, nch * R], bf16c)
        nc.sync.dma_start(bd_t[:], d_bd.ap())
        id_t = cpool.tile([128, 128], mybir.dt.bfloat16)
        nc.sync.dma_start(id_t[:], d_id.ap())

        # bias tiles for ACT ops (float bias needs a pre-registered const AP)
        bias0 = cpool.tile([128, 1], f32)
        nc.vector.memset(bias0[:], 0.0)
        bias_hpi = cpool.tile([128, 1], f32)
        nc.vector.memset(bias_hpi[:], float(np.pi) / 2.0)

        # ---- load inputs: X[p, pair*nch + j] = x[pair, p*nch + j] ----
        rij_t = gpool.tile([128, W], f32)
        rik_t = gpool.tile([128, W], f32)
        rjk_t = gpool.tile([128, W], f32)
        PPC0 = PP // 4
        for c in range(4):
            for dst, src in ((rij_t, d_rij), (rik_t, d_rik), (rjk_t, d_rjk)):
                d3 = dst[:].rearrange("p (pair j) -> p pair j", j=nch)
                s3 = src.ap().rearrange("pair (p j) -> p pair j", j=nch)
                nc.sync.dma_start(d3[:, c * PPC0:(c + 1) * PPC0, :],
                                  s3[:, c * PPC0:(c + 1) * PPC0, :])

        # ---- global elementwise stage, split into 4 column chunks for overlap ----
        tij2 = gpool.tile([128, W], f32)
        tik2 = gpool.tile([128, W], f32)
        tjk2 = gpool.tile([128, W], f32)
        den = gpool.tile([128, W], f32)
        lnd = gpool.tile([128, W], f32)
        rden = gpool.tile([128, W], f32)
        s12 = gpool.tile([128, W], f32)
        num = gpool.tile([128, W], f32)
        s2g = gpool.tile([128, W], f32)
        s2a = gpool.tile([128, W], f32)
        ct = gpool.tile([128, W], f32)
        s1 = gpool.tile([128, W], f32)
        s1a = gpool.tile([128, W], f32)
        c1 = gpool.tile([128, W], f32)
        c2 = gpool.tile([128, W], f32)
        c3 = gpool.tile([128, W], f32)
        p12 = gpool.tile([128, W], f32)
        p2 = gpool.tile([128, W], f32)
        cm = gpool.tile([128, W], f32)
        u1 = gpool.tile([128, W], f32)
        v1 = gpool.tile([128, W], f32)
        u2 = gpool.tile([128, W], f32)
        v2 = gpool.tile([128, W], f32)
        u4 = gpool.tile([128, W], f32)
        v4 = gpool.tile([128, W], f32)
        u8 = gpool.tile([128, W], f32)
        v8 = gpool.tile([128, W], f32)
        s1hi = gpool.tile([128, W], mybir.dt.bfloat16)
        s1lo = gpool.tile([128, W], mybir.dt.bfloat16)
        s2hi = gpool.tile([128, W], mybir.dt.bfloat16)
        s2lo = gpool.tile([128, W], mybir.dt.bfloat16)
        bf16 = mybir.dt.bfloat16
        pall = gpool.tile([128, F * W], bf16)
        PI = float(np.pi)

        def glob_chunk(lo, hi):
            sl = slice(lo, hi)
            nc.gpsimd.tensor_tensor(tij2[:, sl], rij_t[:, sl], rij_t[:, sl], Alu.mult)
            nc.gpsimd.tensor_tensor(tik2[:, sl], rik_t[:, sl], rik_t[:, sl], Alu.mult)
            nc.vector.tensor_tensor(tjk2[:, sl], rjk_t[:, sl], rjk_t[:, sl], Alu.mult)
            nc.vector.tensor_tensor(s12[:, sl], tij2[:, sl], tik2[:, sl], Alu.add)
            nc.vector.scalar_tensor_tensor(num[:, sl], tjk2[:, sl], -1.0, s12[:, sl],
                                           Alu.mult, Alu.add)
            nc.vector.tensor_tensor(s2a[:, sl], s12[:, sl], tjk2[:, sl], Alu.add)
            nc.vector.tensor_scalar(s2g[:, sl], s2a[:, sl], GAMMA, None, Alu.mult)
            nc.vector.tensor_tensor(ct[:, sl], num[:, sl], rden[:, sl], Alu.mult)
            nc.gpsimd.tensor_tensor(s1a[:, sl], rij_t[:, sl], rik_t[:, sl], Alu.add)
            nc.gpsimd.tensor_tensor(s1[:, sl], s1a[:, sl], rjk_t[:, sl], Alu.add)
            nc.gpsimd.tensor_copy(s1hi[:, sl], s1[:, sl])
            nc.vector.tensor_tensor(s1lo[:, sl], s1[:, sl], s1hi[:, sl], Alu.subtract)
            nc.gpsimd.tensor_copy(s2hi[:, sl], s2g[:, sl])
            nc.vector.tensor_tensor(s2lo[:, sl], s2g[:, sl], s2hi[:, sl], Alu.subtract)
            nc.vector.tensor_tensor(p12[:, sl], c1[:, sl], c2[:, sl], Alu.mult)
            nc.vector.tensor_tensor(p2[:, sl], p12[:, sl], c3[:, sl], Alu.mult)
            nc.vector.scalar_tensor_tensor(cm[:, sl], p2[:, sl], 2.0, p2[:, sl], Alu.mult, Alu.mult)
            nc.vector.tensor_scalar(u1[:, sl], ct[:, sl], -0.5, 0.5, Alu.mult, Alu.add)
            nc.vector.tensor_scalar(v1[:, sl], u1[:, sl], -1.0, 1.0, Alu.mult, Alu.add)
            nc.gpsimd.tensor_tensor(u2[:, sl], u1[:, sl], u1[:, sl], Alu.mult)
            nc.vector.tensor_tensor(v2[:, sl], v1[:, sl], v1[:, sl], Alu.mult)
            nc.gpsimd.tensor_tensor(u4[:, sl], u2[:, sl], u2[:, sl], Alu.mult)
            nc.vector.tensor_tensor(v4[:, sl], v2[:, sl], v2[:, sl], Alu.mult)
            nc.gpsimd.tensor_tensor(u8[:, sl], u4[:, sl], u4[:, sl], Alu.mult)
            nc.vector.tensor_tensor(v8[:, sl], v4[:, sl], v4[:, sl], Alu.mult)
            for fi, pw in enumerate((u1, u2, u4, u8, v1, v2, v4, v8)):
                eng = nc.vector if fi % 2 == 0 else nc.gpsimd
                eng.tensor_tensor(pall[:, fi * W + lo:fi * W + hi], pw[:, sl], cm[:, sl], Alu.mult)

        pall_s = pall[:].rearrange("p (f col) -> p col f", f=F)

        # chunked transcendentals, emission grouped by ACT table set (2 loads):
        # all sins (chunk order), then all ln/exp (set shared with pair exps)
        CW = W // 4
        for c in range(4):
            sl = slice(c * CW, (c + 1) * CW)
            nc.scalar.activation(c1[:, sl], rij_t[:, sl], Act.Sin, scale=PI / 10.0, bias=bias_hpi[:])
            nc.scalar.activation(c2[:, sl], rik_t[:, sl], Act.Sin, scale=PI / 10.0, bias=bias_hpi[:])
            nc.scalar.activation(c3[:, sl], rjk_t[:, sl], Act.Sin, scale=PI / 10.0, bias=bias_hpi[:])
        for c in range(4):
            sl = slice(c * CW, (c + 1) * CW)
            nc.vector.tensor_tensor(den[:, sl], rij_t[:, sl], rik_t[:, sl], Alu.mult)
            nc.scalar.activation(lnd[:, sl], den[:, sl], Act.Ln, scale=2.0, bias=bias0[:])
            nc.scalar.activation(rden[:, sl], lnd[:, sl], Act.Exp, scale=-1.0, bias=bias0[:])
        # ---- interleaved transposed source for per-pair u-matmuls ----
        src_t = gpool.tile([128, NB * 128], mybir.dt.bfloat16)
        src3 = src_t[:, 0:NV].rearrange("p (pair k) -> p pair k", k=KRP)
        nc.vector.memset(src3[:, :, 5 * nch:5 * nch + 2], 1.0)
        ts_t = gpool.tile([128, NB * 128], mybir.dt.bfloat16)
        s1h3 = s1hi[:].rearrange("p (pair j) -> p pair j", j=nch)
        s1l3 = s1lo[:].rearrange("p (pair j) -> p pair j", j=nch)
        s2h3 = s2hi[:].rearrange("p (pair j) -> p pair j", j=nch)
        s2l3 = s2lo[:].rearrange("p (pair j) -> p pair j", j=nch)
        NCHK = 4
        PPC = PP // NCHK           # pairs per chunk
        for c in range(NCHK):
            glob_chunk(c * PPC * nch, (c + 1) * PPC * nch)
            pr = slice(c * PPC, (c + 1) * PPC)
            nc.gpsimd.tensor_copy(src3[:, pr, 0:nch], s1h3[:, pr, :])
            nc.gpsimd.tensor_copy(src3[:, pr, nch:2 * nch], s1l3[:, pr, :])
            nc.gpsimd.tensor_copy(src3[:, pr, 2 * nch:3 * nch], s1h3[:, pr, :])
            nc.gpsimd.tensor_copy(src3[:, pr, 3 * nch:4 * nch], s2h3[:, pr, :])
            nc.gpsimd.tensor_copy(src3[:, pr, 4 * nch:5 * nch], s2l3[:, pr, :])
            for b in range(c * NB // NCHK, (c + 1) * NB // NCHK):
                pst = ptpool.tile([128, 128], mybir.dt.bfloat16, name=f"pst{b}", tag="pst")
                nc.tensor.transpose(pst[:], src_t[:, b * 128:(b + 1) * 128], id_t[:])
                nc.vector.tensor_copy(ts_t[:, b * 128:(b + 1) * 128], pst[:])

        # ---- per-pair: u-matmul (f32r), exp->bf16, col-tiled contraction ----
        outs_t = gpool.tile([R, PP * F], f32)
        sel_t = cpool.tile([128, R], f32)
        nc.sync.dma_start(sel_t[:], d_sel.ap())
        # chunk j -> col-group b = j % 4; groups accumulate chunks {b, b+4, b+8}
        bchunks = [[j for j in range(nch) if j % 4 == b] for b in range(4)]
        for g in range(PP // 8):
            pc = pcpool.tile([128, 8 * F], f32)
            for h in range(4):  # 2 pairs per psu tile (2 PSUM banks)
                psu = pupool.tile([128, 1024], f32)
                rad = ppool.tile([128, 2 * nch * R], bf16, name=f"rad{g}_{h}", tag="rad", bufs=6)
                for e in range(2):
                    pair = g * 8 + h * 2 + e
                    blk, p0 = pair // 2, 64 * (pair % 2)
                    nc.tensor.matmul(psu[:, e * 512:e * 512 + nch * R],
                                     ts_t[p0:p0 + KR, blk * 128:(blk + 1) * 128],
                                     bd_t[p0:p0 + KR, :],
                                     start=True, stop=True, tile_position=(p0, 0))
                nc.scalar.activation(
                    rad[:].rearrange("p (e c) -> p e c", e=2),
                    psu[:].rearrange("p (e c) -> p e c", e=2)[:, :, 0:nch * R],
                    Act.Exp, bias=bias0[:])
                for e in range(2):
                    pair = g * 8 + h * 2 + e
                    q = h * 2 + e
                    for b in range(4):
                        for ji, j in enumerate(bchunks[b]):
                            nc.tensor.matmul(
                                pc[32 * b:32 * b + 32, q * F:(q + 1) * F],
                                rad[:, (e * nch + j) * R:(e * nch + j + 1) * R],
                                pall_s[:, pair * nch + j, :],
                                start=(ji == 0), stop=(ji == len(bchunks[b]) - 1),
                                tile_position=(0, 32 * b),
                            )
            # combine 4 col-group partials: out[r, qf] = sum_b pc[32b+r, qf]
            sb = ppool.tile([128, 8 * F], f32, name=f"sb{g}", tag="sb")
            nc.vector.tensor_copy(sb[:], pc[:])
            ps2 = ps2pool.tile([R, 8 * F], f32, name=f"ps2_{g}", tag="pst")
            nc.tensor.matmul(ps2[:], sel_t[:], sb[:], start=True, stop=True)
            nc.vector.tensor_copy(outs_t[:, g * 8 * F:(g + 1) * 8 * F], ps2[:])

        # out[pair, r*8+f] = outs_t[r, pair*8+f], one DMA per 8-pair group
        d_out3 = d_out.ap().rearrange("pair (r f) -> r pair f", f=F)
        o_t3 = outs_t[:].rearrange("r (pair f) -> r pair f", f=F)
        for g in range(PP // 8):
            nc.sync.dma_start(d_out3[:, g * 8:(g + 1) * 8, :],
                              o_t3[:, g * 8:(g + 1) * 8, :])

    nc.compile()
    return nc


def _prep(r_ij, r_ik, r_jk, offsets, triple_masks):
    """Host-side shard + compact + pad. Returns (in_maps, nch)."""
    B, A, N = r_ij.shape
    P = B * A
    rij = np.ascontiguousarray(r_ij, dtype=np.float32).reshape(P, N)
    rik = np.ascontiguousarray(r_ik, dtype=np.float32).reshape(P, N)
    rjk = np.ascontiguousarray(r_jk, dtype=np.float32).reshape(P, N)
    m = (np.asarray(triple_masks).reshape(P, N) != 0)

    counts = m.sum(axis=1)
    npad = max(128, int(-(-max(1, counts.max()) // 128) * 128))
    nch = npad // 128

    cij = np.full((P, npad), 5.0, dtype=np.float32)
    cik = np.full((P, npad), 5.0, dtype=np.float32)
    cjk = np.full((P, npad), 5.0, dtype=np.float32)
    for p in range(P):
        idx = np.nonzero(m[p])[0]
        k = idx.size
        cij[p, :k] = rij[p, idx]
        cik[p, :k] = rik[p, idx]
        cjk[p, :k] = rjk[p, idx]

    o = np.asarray(offsets, dtype=np.float32)
    c32 = (2.0 * GAMMA * o).astype(np.float32)       # c_r
    b32 = (-3.0 * GAMMA * o * o).astype(np.float32)  # -12*o_r^2

    import ml_dtypes
    bf = ml_dtypes.bfloat16
    chi = c32.astype(bf).astype(np.float32)
    clo = (c32 - chi).astype(bf)
    bhi = b32.astype(bf).astype(np.float32)
    blo = (b32 - bhi).astype(bf)
    bd = np.zeros((128, nch * R), dtype=bf)
    for g in range(2):
        o = 64 * g
        for j in range(nch):
            bd[o + j, j * R:(j + 1) * R] = chi.astype(bf)
            bd[o + nch + j, j * R:(j + 1) * R] = chi.astype(bf)
            bd[o + 2 * nch + j, j * R:(j + 1) * R] = clo
            bd[o + 3 * nch + j, j * R:(j + 1) * R] = bf(-1.0)
            bd[o + 4 * nch + j, j * R:(j + 1) * R] = bf(-1.0)
        bd[o + 5 * nch, :] = bhi.astype(bf).repeat(1)[0] if False else 0
        for j in range(nch):
            bd[o + 5 * nch, j * R:(j + 1) * R] = bhi.astype(bf)
            bd[o + 5 * nch + 1, j * R:(j + 1) * R] = blo
    ident = np.eye(128, dtype=bf)
    sel = np.tile(np.eye(R, dtype=np.float32), (4, 1))

    in_maps = []
    for c in range(N_CORES):
        lo, hi = c * PP, (c + 1) * PP
        in_maps.append({
            "rij": cij[lo:hi], "rik": cik[lo:hi], "rjk": cjk[lo:hi],
            "bdiag": bd, "ident": ident, "sel": sel,
        })
    return in_maps, nch


def _ensure_ntff_hook():
    """Register the axon NTFF profile hook if the image's antenv lacks it."""
    import types
    try:
        from antenv.axon_hooks import get_axon_ntff_profile_hook  # noqa: F401
        return
    except ImportError:
        pass
    try:
        sys.path.insert(0, "/root/.axon_site")
        from trn_agent_boot.trn_boot import _ntff_profile_via_ctypes
        hook = _ntff_profile_via_ctypes("/opt/axon/libaxon_pjrt.so")
        import antenv
        mod = types.ModuleType("antenv.axon_hooks")
        _holder = {"h": hook}
        mod.set_axon_ntff_profile_hook = lambda h: _holder.update(h=h)
        mod.get_axon_ntff_profile_hook = lambda: _holder["h"]
        sys.modules["antenv.axon_hooks"] = mod
        antenv.axon_hooks = mod
    except Exception:
        pass


def kernel(r_ij, r_ik, r_jk, offsets, triple_masks):
    global LAST_EXEC_NS
    from concourse.bass_utils import run_bass_kernel_spmd
    _ensure_ntff_hook()

    B, A, N = r_ij.shape
    in_maps, nch = _prep(r_ij, r_ik, r_jk, offsets, triple_masks)
    if nch not in _CACHE:
        _CACHE[nch] = _build(nch)
    nc = _CACHE[nch]

    trace = os.environ.get("KERNEL_TRACE", "0") == "1"
    res = run_bass_kernel_spmd(nc, in_maps, core_ids=list(range(N_CORES)),
                               trace=trace)
    LAST_EXEC_NS = res.exec_time_ns
    out = np.concatenate([r["out"] for r in res.results], axis=0)
    return out.reshape(B, A, R * F)
